# revision 12
# baseline (speedup 1.0000x reference)
"""MoE (8 experts, top-2) Trainium2 kernel.

Strategy: expert-parallel across the 8 NeuronCores. The gate (a
4096x1024 @ 1024x8 matmul + top-2 + renormalize, ~0.03% of total FLOPs)
is computed on the host in float64; it only produces routing metadata
(per-token expert ids + combine weights). Each core then runs the full
expert MLP for its expert's tokens:

    y_e = relu(x_e @ W1[e] + b1[e]) @ W2[e] + b2[e]

entirely on device in a fused Bass/Tile kernel (both matmuls, relu and
biases). The host scatters  out[t] = sum_e w_te * y_e[t]  back (the
combine weights are zero for non-selected experts, so routed compute is
mathematically identical to the reference's dense compute).

Device layout (per core, transposed activations so biases are
per-partition):
  MM1:  hT[FF, C] = W1.T @ xT   (+b1, relu)   lhsT = W1 k-tiles
  MM2:  yT[H, C]  = W2.T @ hT   (+b2)         lhsT = W2 k-tiles
with C = per-expert token capacity (padded), all accumulation in fp32
PSUM. Weights are streamed through small SBUF pools; xT and hT stay
SBUF-resident.
"""

import numpy as np

# ---------------------------------------------------------------- config
NUM_EXPERTS = 8
TOP_K = 2
B, S, H = 4, 1024, 1024
FF = 2 * H
T = B * S
P = 128
KH = H // P    # 8 k-tiles over H
KF = FF // P   # 16 k-tiles over FF
CAP_ALIGN = 4  # token capacity alignment (moving dim is arbitrary)
NTILE = 512    # max matmul moving free dim (one PSUM bank of fp32)
MM_DT_NAME = "f16"  # one of: bf16, f16, f32r, f32

PROFILE = False       # set True (from test.py) to trace + record HW time
LAST_EXEC_NS = None
LAST_RESULTS = None

_cache = {}


def _mm_dt():
    import concourse.mybir as mybir
    import ml_dtypes

    return {
        "bf16": (mybir.dt.bfloat16, ml_dtypes.bfloat16),
        "f16": (mybir.dt.float16, np.float16),
        "f32r": (mybir.dt.float32r, np.float32),
        "f32": (mybir.dt.float32, np.float32),
    }[MM_DT_NAME]


def _build(CAP):
    """Build + compile the per-core Bass program (same for all cores)."""
    import concourse.bass as bass
    import concourse.mybir as mybir
    import concourse.tile as tile
    from concourse import bacc

    mm_dt, _ = _mm_dt()
    f32 = mybir.dt.float32
    ts, ds = bass.ts, bass.ds

    nc = bacc.Bacc("TRN2", debug=False, num_devices=NUM_EXPERTS)

    xt_d = nc.dram_tensor("xt_d", [KH, P, CAP], mm_dt, kind="ExternalInput")
    w1_d = nc.dram_tensor("w1_d", [KF, P, KH * P], mm_dt, kind="ExternalInput")
    b1_d = nc.dram_tensor("b1_d", [P, KF], f32, kind="ExternalInput")
    w2_d = nc.dram_tensor("w2_d", [KH, P, KF * P], mm_dt, kind="ExternalInput")
    b2_d = nc.dram_tensor("b2_d", [P, KH], f32, kind="ExternalInput")
    y_d = nc.dram_tensor("y_d", [KH, P, CAP], f32, kind="ExternalOutput")

    # Equal n-tiles: a matmul's weight load (~97ns fp16) hides under the
    # moving-operand stream only if N is large enough; 3x384 beats
    # 512+512+128 (where the N=128 matmuls are weight-load-bound).
    n_splits = -(-CAP // NTILE)
    base = CAP // n_splits
    rem = CAP - base * n_splits
    n_tiles = []
    n0 = 0
    for j in range(n_splits):
        nsz = base + (1 if j < rem else 0)
        n_tiles.append((n0, nsz))
        n0 += nsz

    # DMA ring split: activations (xt) + outputs (y) go on the SP/sync
    # HWDGE ring; weights (w1/w2) on the Activation/scalar ring. The two
    # rings drain independently, so the first matmul only waits for
    # xt[kh=0] + w1[kf=0] (~1MB) instead of all of xt + w1[0] on one FIFO.
    with tile.TileContext(nc) as tc:
        with (
            tc.tile_pool(name="const", bufs=1) as const,
            tc.tile_pool(name="xtp", bufs=1) as xtp,
            tc.tile_pool(name="hp", bufs=1) as hp,
            tc.tile_pool(name="w1p", bufs=6) as w1p,
            tc.tile_pool(name="w2p", bufs=3) as w2p,
            tc.tile_pool(name="yp", bufs=3) as yp,
            tc.tile_pool(name="psp", bufs=2, space="PSUM") as psp,
        ):
            b1t = const.tile([P, KF], f32)
            nc.sync.dma_start(b1t[:], b1_d.ap())
            b2t = const.tile([P, KH], f32)
            nc.sync.dma_start(b2t[:], b2_d.ap())

            # one tile per k-slice so the scheduler can start matmuls as
            # soon as the first slice lands
            xts = []
            for kh in range(KH):
                xk = xtp.tile([P, CAP], mm_dt, tag=f"xt{kh}")
                nc.sync.dma_start(xk[:], xt_d.ap()[kh])
                xts.append(xk)

            h = hp.tile([P, KF, CAP], mm_dt)

            # ---- MM1: hT[kf, :] = relu(W1.T @ xT + b1) ----
            # The first SPLIT_KF kf-chunks are emitted in two half-K
            # phases (same per-group accumulation order, interleaved
            # program order): the A phases only need xt[0:KH/2], giving
            # the DMA an extra ~5us to land the second half of xt during
            # the cold start. Groups stay open across phases
            # (start/stop accumulate flags), using <=6+3 PSUM banks.
            KH2 = KH // 2
            SPLIT_KF = 2
            w1_tiles = {}

            def load_w1(kf):
                # two half-K weight tiles so the first 12 matmuls of a
                # chunk only wait on half the chunk's DMA
                w1a = w1p.tile([P, KH2 * P], mm_dt, tag="w1a", name="w1a")
                nc.scalar.dma_start(w1a[:], w1_d.ap()[kf, :, 0 : KH2 * P])
                w1b = w1p.tile([P, KH2 * P], mm_dt, tag="w1b", name="w1b")
                nc.scalar.dma_start(w1b[:], w1_d.ap()[kf, :, KH2 * P : KH * P])
                return w1a, w1b

            def mm1_phase(kf, acc, nsz, n0, khs):
                w1a, w1b = w1_tiles[kf]
                for kh in khs:
                    wt = w1a if kh < KH2 else w1b
                    nc.tensor.matmul(
                        acc[:, :nsz],
                        wt[:, ts(kh % KH2, P)],
                        xts[kh][:, ds(n0, nsz)],
                        start=(kh == 0),
                        stop=(kh == KH - 1),
                        skip_group_check=True,
                    )

            def mm1_evict(kf, acc, nsz, n0):
                nc.scalar.activation(
                    h[:, kf, ds(n0, nsz)],
                    acc[:, :nsz],
                    mybir.ActivationFunctionType.Relu,
                    bias=b1t[:, kf : kf + 1],
                )

            open_accs = {}
            for kf in range(SPLIT_KF):
                w1_tiles[kf] = load_w1(kf)
                for j, (n0, nsz) in enumerate(n_tiles):
                    acc = psp.tile([P, NTILE], f32, tag=f"acc{kf}_{j}", name=f"acc{kf}_{j}", bufs=1)
                    open_accs[(kf, j)] = acc
                    mm1_phase(kf, acc, nsz, n0, range(KH2))
            for kf in range(SPLIT_KF):
                for j, (n0, nsz) in enumerate(n_tiles):
                    acc = open_accs[(kf, j)]
                    mm1_phase(kf, acc, nsz, n0, range(KH2, KH))
                    mm1_evict(kf, acc, nsz, n0)
            for kf in range(SPLIT_KF, KF):
                w1_tiles[kf] = load_w1(kf)
                for (n0, nsz) in n_tiles:
                    acc = psp.tile([P, NTILE], f32)
                    mm1_phase(kf, acc, nsz, n0, range(KH))
                    mm1_evict(kf, acc, nsz, n0)

            # ---- MM2: yT[m, :] = W2.T @ hT + b2 ----
            for m in range(KH):
                w2t = w2p.tile([P, KF * P], mm_dt)
                nc.scalar.dma_start(w2t[:], w2_d.ap()[m])
                for (n0, nsz) in n_tiles:
                    acc = psp.tile([P, NTILE], f32)
                    for k in range(KF):
                        nc.tensor.matmul(
                            acc[:, :nsz],
                            w2t[:, ts(k, P)],
                            h[:, k, ds(n0, nsz)],
                            start=(k == 0),
                            stop=(k == KF - 1),
                        )
                    yt = yp.tile([P, NTILE], f32)
                    nc.scalar.activation(
                        yt[:, :nsz],
                        acc[:, :nsz],
                        mybir.ActivationFunctionType.Identity,
                        bias=b2t[:, m : m + 1],
                    )
                    nc.sync.dma_start(y_d.ap()[m, :, ds(n0, nsz)], yt[:, :nsz])

    nc.compile()
    return nc


def _install_profile_shim():
    """Make run_bass_kernel_spmd(trace=True) work under axon in this
    container (the boot-time antenv.axon_hooks install is absent)."""
    import contextlib
    import ctypes
    import sys
    import types

    if "antenv.axon_hooks" in sys.modules:
        return
    so_path = "/opt/axon/libaxon_pjrt.so"
    lib = ctypes.CDLL(so_path)
    if not hasattr(lib, "axon_start_nrt_profile"):
        return
    lib.axon_start_nrt_profile.argtypes = [
        ctypes.POINTER(ctypes.c_int64),
        ctypes.c_size_t,
    ]
    lib.axon_start_nrt_profile.restype = ctypes.c_int64
    lib.axon_stop_nrt_profile.argtypes = [ctypes.c_char_p]
    lib.axon_stop_nrt_profile.restype = ctypes.c_int64

    @contextlib.contextmanager
    def _hook(output_dir, device_ids):
        import jax

        jax.devices()
        if device_ids:
            ids = (ctypes.c_int64 * len(device_ids))(*device_ids)
            rc = lib.axon_start_nrt_profile(ids, len(device_ids))
        else:
            rc = lib.axon_start_nrt_profile(None, 0)
        if rc != 0:
            raise RuntimeError(f"axon_start_nrt_profile rc={rc}")
        try:
            yield
        finally:
            n = lib.axon_stop_nrt_profile(str(output_dir).encode())
            print(f"ntff profile: {n} file(s) in {output_dir}", file=sys.stderr)

    mod = types.ModuleType("antenv.axon_hooks")
    mod.get_axon_ntff_profile_hook = lambda: _hook
    mod.set_axon_ntff_profile_hook = lambda h: None
    sys.modules["antenv.axon_hooks"] = mod

    import concourse.bass_utils as bu

    bu.upload_artifacts = lambda tmpdir: str(tmpdir)


# ---------------------------------------------------------------- host side

def _route(xf, Wg, bg):
    """Top-2 routing on host, float64 scoring. Returns (top2 [T,2] int,
    w [T,2] float32 renormalized combine weights)."""
    logits = xf.astype(np.float64) @ Wg.astype(np.float64) + bg.astype(np.float64)
    top2 = np.argsort(-logits, axis=-1, kind="stable")[:, :TOP_K]
    lv = np.take_along_axis(logits, top2, axis=1)
    lv = lv - lv.max(axis=1, keepdims=True)
    ev = np.exp(lv)
    w = ev / ev.sum(axis=1, keepdims=True)
    return top2, w.astype(np.float32)


def _prep_weights(W1, b1, W2, b2, np_dt):
    """Per-expert DRAM layouts for the device program."""
    per_expert = []
    for e in range(NUM_EXPERTS):
        w1g = (
            W1[e]
            .reshape(KH, P, KF, P)
            .transpose(2, 1, 0, 3)
            .reshape(KF, P, KH * P)
            .astype(np_dt)
        )
        w2g = (
            W2[e]
            .reshape(KF, P, KH, P)
            .transpose(2, 1, 0, 3)
            .reshape(KH, P, KF * P)
            .astype(np_dt)
        )
        b1g = np.ascontiguousarray(b1[e].reshape(KF, P).T).astype(np.float32)
        b2g = np.ascontiguousarray(b2[e].reshape(KH, P).T).astype(np.float32)
        per_expert.append((w1g, w2g, b1g, b2g))
    return per_expert


def kernel(x, Wg, bg, W1, b1, W2, b2):
    global LAST_EXEC_NS, LAST_RESULTS

    x = np.asarray(x, dtype=np.float32)
    Wg = np.asarray(Wg, dtype=np.float32)
    bg = np.asarray(bg, dtype=np.float32)
    W1 = np.asarray(W1, dtype=np.float32)
    b1 = np.asarray(b1, dtype=np.float32)
    W2 = np.asarray(W2, dtype=np.float32)
    b2 = np.asarray(b2, dtype=np.float32)

    _, np_dt = _mm_dt()
    if PROFILE:
        _install_profile_shim()

    from concourse.bass_utils import run_bass_kernel_spmd

    xf = x.reshape(T, H)
    top2, w = _route(xf, Wg, bg)

    per_expert = _prep_weights(W1, b1, W2, b2, np_dt)

    # token lists per expert
    idx_list = []
    wgt_list = []
    for e in range(NUM_EXPERTS):
        mask = top2 == e  # [T, 2]
        idx = np.where(mask.any(axis=1))[0]
        slot = mask[idx, 1].astype(np.int64)  # 0 if slot0, 1 if slot1
        idx_list.append(idx)
        wgt_list.append(w[idx, slot])

    out = np.zeros((T, H), dtype=np.float32)
    max_count = max(len(i) for i in idx_list)
    # capacity: fit the hottest expert exactly (aligned), bounded so a
    # pathological distribution falls back to multiple rounds
    CAP = min(2048, max(512, -(-max_count // CAP_ALIGN) * CAP_ALIGN))
    if CAP not in _cache:
        _cache[CAP] = _build(CAP)
    nc = _cache[CAP]
    n_rounds = max(1, -(-max_count // CAP))

    for r in range(n_rounds):
        in_maps = []
        chunk_idx = []
        for e in range(NUM_EXPERTS):
            idx = idx_list[e][r * CAP : (r + 1) * CAP]
            chunk_idx.append(idx)
            c = len(idx)
            xe = np.zeros((H, CAP), dtype=np_dt)
            if c:
                xe[:, :c] = xf[idx].T.astype(np_dt)
            w1g, w2g, b1g, b2g = per_expert[e]
            in_maps.append(
                {
                    "xt_d": xe.reshape(KH, P, CAP),
                    "w1_d": w1g,
                    "b1_d": b1g,
                    "w2_d": w2g,
                    "b2_d": b2g,
                }
            )
        res = run_bass_kernel_spmd(
            nc,
            in_maps,
            core_ids=list(range(NUM_EXPERTS)),
            trace=bool(PROFILE),
        )
        if PROFILE:
            LAST_EXEC_NS = res.exec_time_ns
            LAST_RESULTS = res
        for e in range(NUM_EXPERTS):
            idx = chunk_idx[e]
            c = len(idx)
            if not c:
                continue
            yT = res.results[e]["y_d"].reshape(H, CAP)  # [H, CAP]
            we = wgt_list[e][r * CAP : (r + 1) * CAP]
            out[idx] += we[:, None] * yT[:, :c].T

    return out.reshape(B, S, H)
